# revision 8
# baseline (speedup 1.0000x reference)
"""Self-attention (channel attention) kernel for Trainium2, 8-core SPMD.

Problem: x (2,16,16,16,64) fp32 -> q = x.reshape(B=2, N=4096, C=64)
  energy = q @ q^T  (per batch, N x N)
  attn = softmax(energy, axis=-1)
  out = gamma * (attn @ q) + x

Sharding: each of the 8 cores computes 512 q-rows of BOTH batches
(core c handles rows [512c, 512c+512)). Each core receives the full x
(as keys, pre-cast to bf16 on the host to halve HBM traffic) plus its
fp32 q-slice (for the residual), and returns its (2, 512, 64) slab.

v3 pipeline per key chunk j (32 chunks of 128 keys):
  PE : transpose K-chunk [128 keys, (b,c)] -> psum bf16   (hoisted +8
       chunks ahead of use so the kt copies never wait behind exp ops
       in the DVE queue)
  DVE: copy psum -> kt[:, j, :]
  PE : S pair, row-tiled both batches -> s_ps fp32 psum
  exp: ACT chunks: pt = Exp(s - 64) -> bf16
       DVE chunks: pt16 = sat_u16(s*184.665 + 4437.3)
         = bf16 bits of 2^((s-64)*log2 e) (Schraudolph; fp32->uint16
         saturates negatives to +0, verified on HW; softmax
         renormalization makes the ~2% weight error invisible)
  PE : PV pair [K|1]^T @ P^T accumulate -> [65, 512] psum

Exp work splits ~2:1 ACT:DVE. All DMA triggers ride the sync ring so
the scalar queue stays pure exp. Epilogue: PE transpose [65,128] tiles,
DVE recip+scale, GpSimd residual add, one output DMA per batch.
"""

import sys

try:
    import concourse  # noqa: F401
except ImportError:
    sys.path.insert(0, "/opt/trn_rl_repo")

import numpy as np

N_CORES = 8
B = 2
N = 4096
C = 64
QROWS = N // N_CORES        # 512 q rows per core (per batch)
NT = N // 128               # 32 key tiles
QT_TILES = QROWS // 128     # 4 q tiles

DVE_CHUNKS = frozenset(j for j in range(NT) if j % 3 == 2)  # 10 chunks
LOOKAHEAD = 8               # kt chunks built this far ahead of use

LOG2E_128 = 184.6650390625                 # 128 * log2(e)
SCHRAU_BIAS = 16256.0 - 64.0 * LOG2E_128   # +4437.3...

_CACHE = {}


def _build_program():
    import concourse.bacc as bacc
    import concourse.tile as tile
    from concourse import mybir

    F32 = mybir.dt.float32
    BF16 = mybir.dt.bfloat16
    U16 = mybir.dt.uint16
    EXP = mybir.ActivationFunctionType.Exp
    MULT = mybir.AluOpType.mult
    ADD = mybir.AluOpType.add

    nc = bacc.Bacc("TRN2", target_bir_lowering=False, debug=False)

    xkb_dram = nc.dram_tensor("xkb", [B, N, C], BF16, kind="ExternalInput")
    xq_dram = nc.dram_tensor("xq", [B, QROWS, C], F32, kind="ExternalInput")
    gam_dram = nc.dram_tensor("gam", [128, 1], F32, kind="ExternalInput")
    ident_dram = nc.dram_tensor("ident", [128, 128], F32, kind="ExternalInput")
    out_dram = nc.dram_tensor("out", [B, QROWS, C], F32, kind="ExternalOutput")

    with tile.TileContext(nc) as tc:
        with (
            tc.tile_pool(name="singles", bufs=1) as singles,
            tc.tile_pool(name="ptp", bufs=4) as ptp,
            tc.tile_pool(name="misc", bufs=8) as misc,
            tc.tile_pool(name="spsum", bufs=2, space="PSUM") as spsum,
            tc.tile_pool(name="trpsum", bufs=2, space="PSUM") as trpsum,
            tc.tile_pool(name="pvpsum", bufs=1, space="PSUM") as pvpsum,
        ):
            ident = singles.tile([128, 128], F32)
            gam = singles.tile([128, 1], F32)
            neg64 = singles.tile([128, 1], F32)
            warm = singles.tile([128, 1], F32)
            xq_nat = singles.tile([128, QT_TILES, B, C], F32)
            knat_bf = singles.tile([128, NT, B, C], BF16)
            kbf65 = singles.tile([128, NT, B, C + 1], BF16)
            kt = singles.tile([128, NT, 128], BF16)
            qt = singles.tile([128, QROWS], BF16)
            ident_bf = singles.tile([128, 128], BF16)
            xq_bf = singles.tile([128, QT_TILES, B, C], BF16)
            obuf = singles.tile([128, B, QT_TILES, C], F32)

            GRP = 8  # key tiles per DMA trigger (per batch)
            NGRP = NT // GRP

            def dma_group(g, lo=0):
                rows = slice(128 * (GRP * g + lo), 128 * GRP * (g + 1))
                for b in range(B):
                    nc.sync.dma_start(
                        out=knat_bf[:, GRP * g + lo : GRP * (g + 1), b, :],
                        in_=xkb_dram.ap()[b, rows, :].rearrange(
                            "(t p) c -> p t c", p=128
                        ),
                    )

            def cast_group(g, lo=0):
                # [K | ones] PV stationary gets its K part from knat_bf
                sl = slice(GRP * g + lo, GRP * (g + 1))
                nc.vector.tensor_copy(
                    kbf65[:, sl, :, 0:C], knat_bf[:, sl, :, :]
                )

            def build_kt(j):
                tr = trpsum.tile([128, 128], BF16, tag="trb")
                nc.tensor.transpose(tr[:], knat_bf[:, j, :, :], ident_bf[:])
                nc.vector.tensor_copy(kt[:, j, :], tr[:])

            # constants first (exp table preload has no DMA deps)
            nc.vector.memset(warm[:], 0.0)
            nc.scalar.activation(warm[:], warm[:], EXP)
            nc.vector.memset(neg64[:], -64.0)
            ones_bf = singles.tile([128, 1], BF16)
            nc.vector.memset(ones_bf[:], 1.0)
            nc.vector.tensor_copy(
                kbf65[:, :, :, C : C + 1],
                ones_bf[:, None, None, :].to_broadcast([128, NT, B, 1]),
            )
            wseed = singles.tile([128, 128], BF16)
            nc.vector.memset(wseed[:], 1.0)

            pv_psA = pvpsum.tile([C + 1, QROWS], F32, tag="pva")
            pv_psB = pvpsum.tile([C + 1, QROWS], F32, tag="pvb")
            pv_ps = [pv_psA, pv_psB]

            # DMAs on the sync ring: q-slice first (qt gates the loop), then
            # key chunk 0 alone (chunk 0's transpose starts early), ident/gam,
            # then the first key groups
            nc.sync.dma_start(
                out=xq_nat[:, :, 0, :],
                in_=xq_dram.ap()[0].rearrange("(t p) c -> p t c", p=128),
            )
            nc.sync.dma_start(
                out=xq_nat[:, :, 1, :],
                in_=xq_dram.ap()[1].rearrange("(t p) c -> p t c", p=128),
            )
            nc.sync.dma_start(
                out=knat_bf[:, 0:1, 0, :],
                in_=xkb_dram.ap()[0, 0:128, :][None].rearrange("o p c -> p o c"),
            )
            nc.sync.dma_start(
                out=knat_bf[:, 0:1, 1, :],
                in_=xkb_dram.ap()[1, 0:128, :][None].rearrange("o p c -> p o c"),
            )
            nc.sync.dma_start(out=ident[:], in_=ident_dram.ap())
            nc.sync.dma_start(out=gam[:], in_=gam_dram.ap())
            dma_group(0, lo=1)
            dma_group(1)

            # PE warm-up burst so the clock gate opens during the DMA wait
            for w in range(6):
                nc.tensor.matmul(
                    pv_ps[w % 2][:, 0:256], wseed[:, 0 : C + 1],
                    wseed[:, None, :].to_broadcast([128, 2, 128]),
                    start=True, stop=True,
                )

            nc.vector.tensor_copy(ident_bf[:], ident[:])
            nc.vector.tensor_copy(xq_bf[:], xq_nat[:])

            # Q^T via one [128,128] PE transpose per q tile
            for t in range(QT_TILES):
                tr = trpsum.tile([128, 128], BF16, tag="trb")
                nc.tensor.transpose(tr[:], xq_bf[:, t, :, :], ident_bf[:])
                nc.vector.tensor_copy(qt[:, 128 * t : 128 * t + 128], tr[:])

            cast_group(0)
            for j in range(LOOKAHEAD):
                build_kt(j)
            dma_group(2)
            cast_group(1)

            # software-pipelined main loop
            LAG = 2
            pt_q = {}
            for j in range(NT + LAG):
                if j < NT:
                    if j % GRP == 0:
                        if j // GRP + 3 < NGRP:
                            dma_group(j // GRP + 3)
                        if j // GRP + 2 < NGRP:
                            cast_group(j // GRP + 2)
                    if j + LOOKAHEAD < NT:
                        build_kt(j + LOOKAHEAD)

                    # S chunk: [128 keys, b, 512 qrows] fp32 psum
                    s_ps = spsum.tile([128, B, QROWS], F32, tag="s")
                    for b in range(B):
                        nc.tensor.matmul(
                            s_ps[:, b, :],
                            kt[64 * b : 64 * b + 64, j, :],
                            qt[64 * b : 64 * b + 64, :],
                            start=True,
                            stop=True,
                            tile_position=(64 * b, 0),
                        )

                    # P^T = exp(S^T - 64) -> bf16 (two engines)
                    pt_t = ptp.tile([128, B, QROWS], BF16, tag="pt")
                    if j in DVE_CHUNKS:
                        nc.vector.tensor_scalar(
                            pt_t[:].bitcast(U16), s_ps[:],
                            LOG2E_128, SCHRAU_BIAS, MULT, ADD,
                        )
                    else:
                        nc.scalar.activation(pt_t[:], s_ps[:], EXP, bias=neg64[:])
                    pt_q[j] = pt_t

                if j >= LAG:
                    jj = j - LAG
                    pt_prev = pt_q.pop(jj)
                    for b in range(B):
                        nc.tensor.matmul(
                            pv_ps[b][:, :],
                            kbf65[:, jj, b, :],
                            pt_prev[:, b, :],
                            start=(jj == 0),
                            stop=(jj == NT - 1),
                        )

            # ---- epilogue ----
            # pv_ps[b] rows 0..63 = O^T (unnormalized), row 64 = row sums
            ovs = {}
            for b in range(B):
                ovs[b] = singles.tile([C + 1, QROWS], F32, tag=f"ov{b}", name=f"ov{b}")
                nc.vector.tensor_copy(ovs[b][:], pv_ps[b][:, :])
            for b in range(B):
                for t in range(QT_TILES):
                    cols = slice(128 * t, 128 * t + 128)
                    o_tr = spsum.tile([128, C + 1], F32, tag="s")
                    nc.tensor.transpose(
                        o_tr[:], ovs[b][:, cols], ident[0 : C + 1, 0 : C + 1]
                    )
                    recip = misc.tile([128, 1], F32, tag="recip")
                    nc.vector.reciprocal(recip[:], o_tr[:, C : C + 1])
                    scale = misc.tile([128, 1], F32, tag="scale")
                    nc.vector.tensor_tensor(scale[:], recip[:], gam[:], MULT)
                    nc.vector.tensor_tensor(
                        obuf[:, b, t, :],
                        o_tr[:, 0:C],
                        scale[:].to_broadcast([128, C]),
                        MULT,
                    )
                    nc.gpsimd.tensor_tensor(
                        obuf[:, b, t, :], obuf[:, b, t, :], xq_nat[:, t, b, :],
                        ADD,
                    )
            for b in range(B):
                nc.sync.dma_start(
                    out=out_dram.ap()[b].rearrange("(t p) c -> p t c", p=128),
                    in_=obuf[:, b, :, :],
                )

    nc.compile()
    return nc


def _get_nc():
    if "nc" not in _CACHE:
        _CACHE["nc"] = _build_program()
    return _CACHE["nc"]


def kernel(x, gamma, _trace=False, _trace_kwargs=None):
    import ml_dtypes

    from concourse.bass_utils import run_bass_kernel_spmd

    x = np.asarray(x, dtype=np.float32)
    gamma = np.asarray(gamma, dtype=np.float32)
    shape_in = x.shape
    xk = np.ascontiguousarray(x.reshape(B, N, C))
    xkb = xk.astype(ml_dtypes.bfloat16)
    gam = np.full((128, 1), float(gamma.reshape(-1)[0]), dtype=np.float32)
    ident = np.eye(128, dtype=np.float32)

    nc = _get_nc()
    in_maps = [
        {
            "xkb": xkb,
            "xq": np.ascontiguousarray(xk[:, QROWS * c : QROWS * (c + 1), :]),
            "gam": gam,
            "ident": ident,
        }
        for c in range(N_CORES)
    ]
    res = run_bass_kernel_spmd(
        nc,
        in_maps,
        core_ids=list(range(N_CORES)),
        trace=_trace,
        **(_trace_kwargs or {}),
    )
    out = np.empty((B, N, C), dtype=np.float32)
    for c in range(N_CORES):
        out[:, QROWS * c : QROWS * (c + 1), :] = res.results[c]["out"]
    if _trace:
        _CACHE["last_results"] = res
    return out.reshape(shape_in)


# revision 10
# speedup vs baseline: 1.0588x; 1.0588x over previous
"""Self-attention (channel attention) kernel for Trainium2, 8-core SPMD.

Problem: x (2,16,16,16,64) fp32 -> q = x.reshape(B=2, N=4096, C=64)
  energy = q @ q^T  (per batch, N x N)
  attn = softmax(energy, axis=-1)
  out = gamma * (attn @ q) + x

Sharding: each of the 8 cores computes 512 q-rows of BOTH batches
(core c handles rows [512c, 512c+512)). Each core receives the full x
as keys (pre-cast to bf16 on the host, halving HBM traffic) plus its
fp32 q-slice for the residual, and returns its (2, 512, 64) slab.

v4 structure (PE cycles/chunk: S 512 row-tiled + PV 1024 + tr 128):
  - 12 warm-up matmuls on junk SBUF emitted as the FIRST PE
    instructions (no DMA deps) so the clock ramps during the preamble
  - kt built with +8 chunk lookahead; 4 transposes share one psum tile
    and ONE DVE copy (amortizes the psum access penalty)
  - exp split 20 ACT / 12 DVE: ACT chunks Exp(s-64)->bf16; DVE chunks
    sat_u16(s*184.665 + 4437.3) = Schraudolph bf16 bits (fp32->uint16
    saturates negatives to +0; softmax renormalization hides the ~2%
    weight error - verified numerically and on HW)
  - PV pair [K|1]^T @ P^T, fp32 psum accumulate, LAG=3 behind exp
  - bf16 utility copies (kbf65, xq_bf) on GpSimd; PV-psum drains on ACT
  - all DMA triggers on the sync ring; epilogue ends in one output DMA
    per batch
"""

import sys

try:
    import concourse  # noqa: F401
except ImportError:
    sys.path.insert(0, "/opt/trn_rl_repo")

import numpy as np

N_CORES = 8
B = 2
N = 4096
C = 64
QROWS = N // N_CORES        # 512 q rows per core (per batch)
NT = N // 128               # 32 key tiles
QT_TILES = QROWS // 128     # 4 q tiles

DVE_CHUNKS = frozenset(j for j in range(NT) if j % 8 in (2, 4, 7))  # 12
LOOKAHEAD = 8
TRB = 4                     # transposes batched per psum tile / DVE copy
LAG = 3                     # chunks PV trails the exp stage by

LOG2E_128 = 184.6650390625                 # 128 * log2(e)
SCHRAU_BIAS = 16256.0 - 64.0 * LOG2E_128   # +4437.3...

_CACHE = {}


def _build_program():
    import concourse.bacc as bacc
    import concourse.tile as tile
    from concourse import mybir

    F32 = mybir.dt.float32
    BF16 = mybir.dt.bfloat16
    U16 = mybir.dt.uint16
    EXP = mybir.ActivationFunctionType.Exp
    COPY = mybir.ActivationFunctionType.Copy
    MULT = mybir.AluOpType.mult
    ADD = mybir.AluOpType.add

    nc = bacc.Bacc("TRN2", target_bir_lowering=False, debug=False)

    xkb_dram = nc.dram_tensor("xkb", [B, N, C], BF16, kind="ExternalInput")
    xq_dram = nc.dram_tensor("xq", [B, QROWS, C], F32, kind="ExternalInput")
    gam_dram = nc.dram_tensor("gam", [128, 1], F32, kind="ExternalInput")
    ident_dram = nc.dram_tensor("ident", [128, 128], F32, kind="ExternalInput")
    out_dram = nc.dram_tensor("out", [B, QROWS, C], F32, kind="ExternalOutput")

    with tile.TileContext(nc) as tc:
        with (
            tc.tile_pool(name="singles", bufs=1) as singles,
            tc.tile_pool(name="ptp", bufs=6) as ptp,
            tc.tile_pool(name="misc", bufs=8) as misc,
            tc.tile_pool(name="spsum", bufs=2, space="PSUM") as spsum,
            tc.tile_pool(name="trpsum", bufs=2, space="PSUM") as trpsum,
            tc.tile_pool(name="pvpsum", bufs=1, space="PSUM") as pvpsum,
        ):
            # warm-up seeds: junk SBUF contents are fine, the results land
            # in psum that is later overwritten with start=True
            junk = singles.tile([128, 128], BF16)
            nc.gpsimd.memset(junk[:], 1.0)
            pv_psA = pvpsum.tile([C + 1, QROWS], F32, tag="pva")
            pv_psB = pvpsum.tile([C + 1, QROWS], F32, tag="pvb")
            pv_ps = [pv_psA, pv_psB]
            for w in range(12):
                nc.tensor.matmul(
                    pv_ps[w % 2][:, 0:256], junk[:, 0 : C + 1],
                    junk[:, None, 0:128].to_broadcast([128, 2, 128]),
                    start=True, stop=True,
                )

            ident = singles.tile([128, 128], F32)
            gam = singles.tile([128, 1], F32)
            neg64 = singles.tile([128, 1], F32)
            warm = singles.tile([128, 1], F32)
            xq_nat = singles.tile([128, QT_TILES, B, C], F32)
            knat_bf = singles.tile([128, NT, B, C], BF16)
            kbf65 = singles.tile([128, NT, B, C + 1], BF16)
            kt = singles.tile([128, NT, 128], BF16)
            qt = singles.tile([128, QROWS], BF16)
            ident_bf = singles.tile([128, 128], BF16)
            xq_bf = singles.tile([128, QT_TILES, B, C], BF16)
            obuf = singles.tile([128, B, QT_TILES, C], F32)

            GRP = 8  # key tiles per DMA trigger (per batch)
            NGRP = NT // GRP

            def dma_group(g, lo=0):
                rows = slice(128 * (GRP * g + lo), 128 * GRP * (g + 1))
                for b in range(B):
                    nc.sync.dma_start(
                        out=knat_bf[:, GRP * g + lo : GRP * (g + 1), b, :],
                        in_=xkb_dram.ap()[b, rows, :].rearrange(
                            "(t p) c -> p t c", p=128
                        ),
                    )

            def cast_group(g, lo=0):
                # [K | ones] PV stationary K-part, bf16->bf16 on GpSimd
                sl = slice(GRP * g + lo, GRP * (g + 1))
                nc.gpsimd.tensor_copy(
                    kbf65[:, sl, :, 0:C], knat_bf[:, sl, :, :]
                )

            def build_kt(j0):
                # TRB chunks -> one psum tile -> one DVE copy
                n = min(TRB, NT - j0)
                tr = trpsum.tile([128, TRB, 128], BF16, tag="trb")
                for i in range(n):
                    nc.tensor.transpose(
                        tr[:, i, :], knat_bf[:, j0 + i, :, :], ident_bf[:]
                    )
                nc.vector.tensor_copy(
                    kt[:, j0 : j0 + n, :], tr[:, 0:n, :]
                )

            # exp table preload + constants (no DMA deps)
            nc.vector.memset(warm[:], 0.0)
            nc.scalar.activation(warm[:], warm[:], EXP)
            nc.vector.memset(neg64[:], -64.0)
            ones_bf = singles.tile([128, 1], BF16)
            nc.vector.memset(ones_bf[:], 1.0)
            nc.vector.tensor_copy(
                kbf65[:, :, :, C : C + 1],
                ones_bf[:, None, None, :].to_broadcast([128, NT, B, 1]),
            )

            # DMAs on the sync ring: q-slice first (qt gates the loop), key
            # chunk 0 alone, ident/gam, then key groups
            nc.sync.dma_start(
                out=xq_nat[:, :, 0, :],
                in_=xq_dram.ap()[0].rearrange("(t p) c -> p t c", p=128),
            )
            nc.sync.dma_start(
                out=xq_nat[:, :, 1, :],
                in_=xq_dram.ap()[1].rearrange("(t p) c -> p t c", p=128),
            )
            nc.sync.dma_start(
                out=knat_bf[:, 0:1, 0, :],
                in_=xkb_dram.ap()[0, 0:128, :][None].rearrange("o p c -> p o c"),
            )
            nc.sync.dma_start(
                out=knat_bf[:, 0:1, 1, :],
                in_=xkb_dram.ap()[1, 0:128, :][None].rearrange("o p c -> p o c"),
            )
            nc.sync.dma_start(out=ident[:], in_=ident_dram.ap())
            nc.sync.dma_start(out=gam[:], in_=gam_dram.ap())
            dma_group(0, lo=1)
            dma_group(1)

            nc.vector.tensor_copy(ident_bf[:], ident[:])
            nc.gpsimd.tensor_copy(xq_bf[:], xq_nat[:])

            # Q^T via one [128,128] PE transpose per q tile (batched copy)
            trq = trpsum.tile([128, TRB, 128], BF16, tag="trb")
            for t in range(QT_TILES):
                nc.tensor.transpose(trq[:, t, :], xq_bf[:, t, :, :], ident_bf[:])
            nc.vector.tensor_copy(
                qt[:].rearrange("p (t x) -> p t x", t=QT_TILES), trq[:]
            )

            cast_group(0)
            for j0 in range(0, LOOKAHEAD, TRB):
                build_kt(j0)
            dma_group(2)
            cast_group(1)

            # software-pipelined main loop
            pt_q = {}
            for j in range(NT + LAG):
                if j < NT:
                    if j % GRP == 0:
                        if j // GRP + 3 < NGRP:
                            dma_group(j // GRP + 3)
                        if j // GRP + 2 < NGRP:
                            cast_group(j // GRP + 2)
                    if (j + LOOKAHEAD) < NT and (j + LOOKAHEAD) % TRB == 0:
                        build_kt(j + LOOKAHEAD)

                    # S chunk: [128 keys, b, 512 qrows] fp32 psum
                    s_ps = spsum.tile([128, B, QROWS], F32, tag="s")
                    for b in range(B):
                        nc.tensor.matmul(
                            s_ps[:, b, :],
                            kt[64 * b : 64 * b + 64, j, :],
                            qt[64 * b : 64 * b + 64, :],
                            start=True,
                            stop=True,
                            tile_position=(64 * b, 0),
                        )

                    # P^T = exp(S^T - 64) -> bf16 (two engines)
                    pt_t = ptp.tile([128, B, QROWS], BF16, tag="pt")
                    if j in DVE_CHUNKS:
                        nc.vector.tensor_scalar(
                            pt_t[:].bitcast(U16), s_ps[:],
                            LOG2E_128, SCHRAU_BIAS, MULT, ADD,
                        )
                    else:
                        nc.scalar.activation(pt_t[:], s_ps[:], EXP, bias=neg64[:])
                    pt_q[j] = pt_t

                if j >= LAG:
                    jj = j - LAG
                    pt_prev = pt_q.pop(jj)
                    for b in range(B):
                        nc.tensor.matmul(
                            pv_ps[b][:, :],
                            kbf65[:, jj, b, :],
                            pt_prev[:, b, :],
                            start=(jj == 0),
                            stop=(jj == NT - 1),
                        )

            # ---- epilogue ----
            # pv_ps[b] rows 0..63 = O^T (unnormalized), row 64 = row sums.
            # psum->SBUF drain on ACT (free after the exp stream ends).
            ovs = {}
            for b in range(B):
                ovs[b] = singles.tile([C + 1, QROWS], F32, tag=f"ov{b}", name=f"ov{b}")
                nc.scalar.activation(ovs[b][:], pv_ps[b][:, :], COPY)
            for b in range(B):
                for t in range(QT_TILES):
                    cols = slice(128 * t, 128 * t + 128)
                    o_tr = spsum.tile([128, C + 1], F32, tag="s")
                    nc.tensor.transpose(
                        o_tr[:], ovs[b][:, cols], ident[0 : C + 1, 0 : C + 1]
                    )
                    recip = misc.tile([128, 1], F32, tag="recip")
                    nc.vector.reciprocal(recip[:], o_tr[:, C : C + 1])
                    scale = misc.tile([128, 1], F32, tag="scale")
                    nc.vector.tensor_tensor(scale[:], recip[:], gam[:], MULT)
                    nc.vector.tensor_tensor(
                        obuf[:, b, t, :],
                        o_tr[:, 0:C],
                        scale[:].to_broadcast([128, C]),
                        MULT,
                    )
                    nc.gpsimd.tensor_tensor(
                        obuf[:, b, t, :], obuf[:, b, t, :], xq_nat[:, t, b, :],
                        ADD,
                    )
            for b in range(B):
                nc.sync.dma_start(
                    out=out_dram.ap()[b].rearrange("(t p) c -> p t c", p=128),
                    in_=obuf[:, b, :, :],
                )

    nc.compile()
    return nc


def _get_nc():
    if "nc" not in _CACHE:
        _CACHE["nc"] = _build_program()
    return _CACHE["nc"]


def kernel(x, gamma, _trace=False, _trace_kwargs=None):
    import ml_dtypes

    from concourse.bass_utils import run_bass_kernel_spmd

    x = np.asarray(x, dtype=np.float32)
    gamma = np.asarray(gamma, dtype=np.float32)
    shape_in = x.shape
    xk = np.ascontiguousarray(x.reshape(B, N, C))
    xkb = xk.astype(ml_dtypes.bfloat16)
    gam = np.full((128, 1), float(gamma.reshape(-1)[0]), dtype=np.float32)
    ident = np.eye(128, dtype=np.float32)

    nc = _get_nc()
    in_maps = [
        {
            "xkb": xkb,
            "xq": np.ascontiguousarray(xk[:, QROWS * c : QROWS * (c + 1), :]),
            "gam": gam,
            "ident": ident,
        }
        for c in range(N_CORES)
    ]
    res = run_bass_kernel_spmd(
        nc,
        in_maps,
        core_ids=list(range(N_CORES)),
        trace=_trace,
        **(_trace_kwargs or {}),
    )
    out = np.empty((B, N, C), dtype=np.float32)
    for c in range(N_CORES):
        out[:, QROWS * c : QROWS * (c + 1), :] = res.results[c]["out"]
    if _trace:
        _CACHE["last_results"] = res
    return out.reshape(shape_in)
